# revision 1
# baseline (speedup 1.0000x reference)
"""TRN2 Bass kernel for AttentionRelPos.

Problem: B=2, T=8, S=196 (14x14), DIM=768, HEADS=12, HD=64.
  qkv = x @ qkv_w.T -> q,k,v [B, 12, 1568, 64]
  attn = softmax(q k^T / 8 + decomposed rel-pos bias)
  out = (attn @ v) heads-concat @ proj_w.T + proj_b

Sharding: 24 (batch, head) pairs -> 3 per core (8 cores). Core c handles
batch c//4, heads 3*(c%4)+[0,1,2]. Each core computes a partial final
projection over its 192 channels; the host sums the 4 partials per batch
(tensor-parallel unshard) and transposes back.

Device-side trick: the decomposed rel-pos bias is folded into the QK
matmul by augmenting the contraction dim from 64 to 100:
  Q'[q] = [q/8, rel_h(q), rel_w(q), rel_t(q)]  (rel_* computed on device)
  K'[k] = [k, onehot_h(k), onehot_w(k), onehot_t(k)]
so S = Q'.K' needs no separate bias pass. Softmax denominators come from a
ones-column appended to V. All heavy matmuls run as float32r (~1.5e-4 rel
err, 4x the fp32 rate).
"""

import os
import sys

for _p in (
    "/root/.axon_site",
    "/root/.axon_site/_ro/trn_rl_repo",
    "/root/.axon_site/_ro/pypackages",
    "/opt/trn_rl_repo",
):
    if os.path.isdir(_p) and _p not in sys.path:
        sys.path.append(_p)

import numpy as np

B, T, HW_, DIM, HEADS, HD = 2, 8, 14, 768, 12, 64
S = HW_ * HW_          # 196
N = T * S              # 1568
NK = 1664              # key count padded to 13*128
KT = 13                # k tiles of 128
QC = 392               # q chunk (196-aligned, 4 per row)
NQC = 4
NAUG = 46              # 14 (h) + 8 (t) + 10 zero pad + 14 (w)
NF = HD + NAUG         # 110 = augmented contraction dim
SCALE = 0.125          # hd ** -0.5
N_CORES = 8
HEADS_PER_CORE = 3

_cached = None


def _build_bass(mm_dt_name="float32r", pt_dt_name="float32r", debug=False, ablate=(),
                reps=1):
    import concourse.bass as bass
    import concourse.mybir as mybir
    import concourse.tile as tile
    from concourse import bacc

    f32 = mybir.dt.float32
    mm_dt = getattr(mybir.dt, mm_dt_name)
    pt_dt = getattr(mybir.dt, pt_dt_name)

    nc = bacc.Bacc("TRN2", target_bir_lowering=False, debug=False,
                   num_devices=N_CORES)

    d_xt = nc.dram_tensor("xt", [DIM, N], f32, kind="ExternalInput").ap()
    d_wt = nc.dram_tensor("wt", [DIM, 576], f32, kind="ExternalInput").ap()
    d_rht = nc.dram_tensor("rht", [HD, 196], f32, kind="ExternalInput").ap()
    d_rwt = nc.dram_tensor("rwt", [HD, 196], f32, kind="ExternalInput").ap()
    d_rtt = nc.dram_tensor("rtt", [HD, 64], f32, kind="ExternalInput").ap()
    d_aug = nc.dram_tensor("aug", [NAUG, NK], f32, kind="ExternalInput").ap()
    d_pwt = nc.dram_tensor("pwt", [192, DIM], f32, kind="ExternalInput").ap()
    d_pb = nc.dram_tensor("pb", [6, 128], f32, kind="ExternalInput").ap()
    d_id = nc.dram_tensor("ident", [128, 128], f32, kind="ExternalInput").ap()
    d_cns = nc.dram_tensor("cns", [128, 110], f32, kind="ExternalInput").ap()
    d_po = nc.dram_tensor("po", [6, NQC, 128, QC], f32, kind="ExternalOutput").ap()
    dbg = {}
    if debug:
        dbg["qt"] = nc.dram_tensor("dbg_qt", [NF, HEADS_PER_CORE, N], f32,
                                   kind="ExternalOutput").ap()
        dbg["kt"] = nc.dram_tensor("dbg_kt", [NF, HEADS_PER_CORE, NK], f32,
                                   kind="ExternalOutput").ap()
        dbg["pt"] = nc.dram_tensor("dbg_pt", [128, KT, QC], f32,
                                   kind="ExternalOutput").ap()
        dbg["ot"] = nc.dram_tensor("dbg_ot", [128, N], f32,
                                   kind="ExternalOutput").ap()

    def bc(ap):
        """View an fp32 dram AP as the matmul dtype (byte-identical load)."""
        return ap.bitcast(mm_dt) if mm_dt != f32 else ap

    with tile.TileContext(nc) as tc:
        with (
            tc.tile_pool(name="const", bufs=1) as cpool,
            tc.tile_pool(name="big", bufs=1) as bpool,
            tc.tile_pool(name="work", bufs=6) as wpool,
            tc.tile_pool(name="stage", bufs=6) as spool,
        ):
          for rep in range(reps):
            # ---------------- constants / inputs ----------------
            p1pool_cm = tc.tile_pool(name=f"p1sbuf{rep}", bufs=1)
            p1pool = p1pool_cm.__enter__()
            xt = p1pool.tile([128, 6, N], mm_dt, tag="xt")
            wt = p1pool.tile([128, 6, 576], mm_dt, tag="wt")
            for c in range(6):
                eng = nc.sync if c % 2 == 0 else nc.scalar
                eng.dma_start(wt[:, c, :], bc(d_wt[c * 128:(c + 1) * 128, :]))
                eng.dma_start(xt[:, c, :], bc(d_xt[c * 128:(c + 1) * 128, :]))
            rht = cpool.tile([HD, 196], mm_dt, tag="rht")
            nc.sync.dma_start(rht[:], bc(d_rht[:]))
            rwt = cpool.tile([HD, 196], mm_dt, tag="rwt")
            nc.scalar.dma_start(rwt[:], bc(d_rwt[:]))
            rtt = cpool.tile([HD, 64], mm_dt, tag="rtt")
            nc.sync.dma_start(rtt[:], bc(d_rtt[:]))
            ident = cpool.tile([128, 128], mm_dt, tag="ident")
            nc.scalar.dma_start(ident[:], bc(d_id[:]))
            pwt0 = cpool.tile([128, DIM], mm_dt, tag="pwt0")
            nc.sync.dma_start(pwt0[:], bc(d_pwt[0:128, :]))
            pwt1 = cpool.tile([64, DIM], mm_dt, tag="pwt1")
            nc.scalar.dma_start(pwt1[:], bc(d_pwt[128:192, :]))
            pb = cpool.tile([128, 6], f32, tag="pb")
            for m in range(6):
                nc.sync.dma_start(pb[:, m:m + 1], d_pb[m, :][:, None])

            # augmented Q'/K' tiles; rows 0:64 filled by QKV phase,
            # rows 64:100 are rel-pos (Q') / one-hot indicators (K')
            qt = bpool.tile([NF, HEADS_PER_CORE, N], mm_dt, tag="qt")
            kt_ = bpool.tile([NF, HEADS_PER_CORE, NK], mm_dt, tag="kt")
            for h in range(HEADS_PER_CORE):
                nc.sync.dma_start(kt_[HD:NF, h, :], bc(d_aug[:]))
                nc.sync.dma_start(kt_[0:HD, h, N:NK], bc(d_cns[0:HD, 0:96]))

            vt01 = p1pool.tile([128, NK], mm_dt, tag="vt01")
            vt2 = p1pool.tile([64, NK], mm_dt, tag="vt2")
            nc.sync.dma_start(vt01[:, N:NK], bc(d_cns[:, 0:96]))
            nc.sync.dma_start(vt2[:, N:NK], bc(d_cns[0:64, 0:96]))

            vp = [bpool.tile([128, KT, HD + 1], pt_dt, tag=f"vp{h}",
                             name=f"vp{h}_{rep}")
                  for h in range(HEADS_PER_CORE)]
            bcp = (lambda ap: ap.bitcast(pt_dt)) if pt_dt != f32 else (lambda ap: ap)
            for h in range(HEADS_PER_CORE):
                # ones column (softmax denominator); zero on the padded k rows
                nc.sync.dma_start(vp[h][:, 0:KT - 1, HD:HD + 1],
                                  bcp(d_cns[:, 96:96 + KT - 1])[:, :, None])
                nc.sync.dma_start(vp[h][:, KT - 1, HD:HD + 1],
                                  bcp(d_cns[:, 108:109]))

            outT01 = bpool.tile([128, N], mm_dt, tag="outT01")
            outT2 = bpool.tile([64, N], mm_dt, tag="outT2")

            # ---------------- phase 1: QKV + rel + V-transpose ----------------
            with tc.tile_pool(name=f"ppsum1{rep}", bufs=2, space="PSUM") as ppsum1:
                # QKV: 5 M-tiles: [q0|k0], [q1|k1], [q2|k2], [v0|v1], [v2]
                for mt in (3, 4, 0, 1, 2):
                    msz = 128 if mt < 4 else 64
                    for qc in range(NQC):
                        ps = ppsum1.tile([128, QC], f32, tag="qkv")
                        sl = slice(qc * QC, (qc + 1) * QC)
                        for c in range(6):
                            nc.tensor.matmul(
                                ps[0:msz, :],
                                wt[:, c, mt * 128:mt * 128 + msz],
                                xt[:, c, sl],
                                start=(c == 0), stop=(c == 5),
                            )
                        if mt < 3:
                            nc.vector.tensor_copy(qt[0:HD, mt, sl], ps[0:64, :])
                            nc.scalar.copy(kt_[0:HD, mt, sl], ps[64:128, :])
                        elif mt == 3:
                            (nc.vector.tensor_copy if qc % 2 else nc.scalar.copy)(
                                vt01[:, sl], ps[:, :])
                        else:
                            (nc.vector.tensor_copy if qc % 2 else nc.scalar.copy)(
                                vt2[:, sl], ps[0:64, :])

                # rel_h / rel_w: 14 groups each, batched over heads+t-blocks
                qt5 = qt[0:HD, :, :].rearrange("p h (t i w) -> p h t i w",
                                               t=T, i=HW_, w=HW_)
                qtr_h = qt[HD:HD + 14, :, :].rearrange(
                    "p h (t i w) -> p h t i w", t=T, i=HW_, w=HW_)
                qtr_w = qt[HD + 32:NF, :, :].rearrange(
                    "p h (t i w) -> p h t i w", t=T, i=HW_, w=HW_)
                for i in range(HW_):
                    ps = ppsum1.tile([14, 336], f32, tag="rel")
                    nc.tensor.matmul(ps[:], rht[:, i * 14:(i + 1) * 14],
                                     qt5[:, :, :, i, :], start=True, stop=True)
                    src = ps[:].rearrange("p (h t w) -> p h t w", h=3, t=T)
                    if i % 2:
                        nc.vector.tensor_copy(qtr_h[:, :, :, i, :], src)
                    else:
                        nc.scalar.copy(qtr_h[:, :, :, i, :], src)
                for j in range(HW_):
                    ps = ppsum1.tile([14, 336], f32, tag="rel")
                    nc.tensor.matmul(ps[:], rwt[:, j * 14:(j + 1) * 14],
                                     qt5[:, :, :, :, j], start=True, stop=True)
                    # dst partitions 78:92 are not 32-aligned (DVE can't);
                    # ScalarE Activation copies allow unaligned partition starts
                    src2 = ps[:].rearrange("p (h t i) -> p h t i", h=3, t=T)
                    nc.scalar.copy(qtr_w[:, :, :, :, j], src2)
                # rel_t: per t-block, two half-blocks of 98 to keep free>=256.
                # dst rows 78:86 are not 32-aligned, so bounce via an aligned
                # staging tile + DMA (contiguous-ish descriptors, cheap)
                qtr_t = qt[HD + 14:HD + 22, :, :]
                for h in range(HEADS_PER_CORE):
                    nc.sync.dma_start(qt[HD + 22:HD + 32, h, :],
                                      bc(d_aug[22:32, 0:N]))
                for t in range(T):
                    ps = ppsum1.tile([8, 2, 512], f32, tag="rel")
                    for half in range(2):
                        c0 = t * S + half * 98
                        nc.tensor.matmul(ps[:, half, 0:294],
                                         rtt[:, t * 8:(t + 1) * 8],
                                         qt[0:HD, :, c0:c0 + 98],
                                         start=True, stop=True)
                    tst = wpool.tile([8, 3, 2, 98], mm_dt, tag="tst")
                    (nc.vector.tensor_copy if t % 2 else nc.scalar.copy)(
                        tst[:].rearrange("p h f w -> p f h w"),
                        ps[:, :, 0:294].rearrange("p f (h w) -> p f h w", h=3))
                    (nc.sync if t % 2 else nc.scalar).dma_start(
                        qtr_t[:, :, t * S:(t + 1) * S],
                        tst[:].rearrange("p h f w -> p h (f w)"))

                # V transpose: vt01 [128, NK] -> per-head V' [k, 64]
                for k in range(KT):
                    sl = slice(k * 128, (k + 1) * 128)
                    ps = ppsum1.tile([128, 128], mm_dt, tag="vtr")
                    nc.tensor.transpose(ps[:], vt01[:, sl], ident[:])
                    (nc.vector.tensor_copy if k % 2 else nc.scalar.copy)(
                        vp[0][:, k, 0:HD], ps[:, 0:64])
                    (nc.scalar.copy if k % 2 else nc.vector.tensor_copy)(
                        vp[1][:, k, 0:HD], ps[:, 64:128])
                    ps2 = ppsum1.tile([128, 128], mm_dt, tag="vtr")
                    nc.tensor.transpose(ps2[:, 0:64], vt2[:, sl], ident[0:64, 0:64])
                    (nc.vector.tensor_copy if k % 2 else nc.scalar.copy)(
                        vp[2][:, k, 0:HD], ps2[:, 0:64])

            if debug and rep == 0:
                for h in range(HEADS_PER_CORE):
                    nc.sync.dma_start(dbg["qt"][:, h, :], qt[:, h, :].bitcast(f32))
                    nc.sync.dma_start(dbg["kt"][:, h, :], kt_[:, h, :].bitcast(f32))

            p1pool_cm.__exit__(None, None, None)

            # ---------------- phase 2: attention + projection ----------------
            with (
                tc.tile_pool(name=f"spsum{rep}", bufs=2, space="PSUM") as spsum,
                tc.tile_pool(name=f"vpsum{rep}", bufs=1, space="PSUM") as vpsum,
                tc.tile_pool(name=f"jpsum{rep}", bufs=1, space="PSUM") as jpsum,
                tc.tile_pool(name=f"ptpool{rep}", bufs=3) as ptpool,
            ):
                groups = [(0, 3), (3, 3), (6, 3), (9, 2), (11, 2)]
                for qc in range(NQC if "p1" not in ablate else 0):
                    sl = slice(qc * QC, (qc + 1) * QC)
                    for h in range(HEADS_PER_CORE):
                        ptt = ptpool.tile([128, KT, QC], pt_dt, tag="pt")
                        for g0, glen in groups:
                            sp = spsum.tile([128, 3, 512], f32, tag="sp")
                            for j in range(glen):
                                k = g0 + j
                                nc.tensor.matmul(
                                    sp[:, j, 0:QC],
                                    kt_[:, h, k * 128:(k + 1) * 128],
                                    qt[:, h, sl],
                                    start=True, stop=True,
                                )
                            nc.scalar.activation(
                                ptt[:, g0:g0 + glen, :], sp[:, 0:glen, 0:QC],
                                bass.mybir.ActivationFunctionType.Exp,
                            )
                        pv_full = vpsum.tile([HD + 1, QC], f32, tag="pv", name="pv")
                        pv = pv_full[:]
                        for k in range(KT):
                            nc.tensor.matmul(pv[:], vp[h][:, k, :], ptt[:, k, :],
                                             start=(k == 0), stop=(k == KT - 1))
                        recip = wpool.tile([1, QC], f32, tag="recip")
                        nc.vector.reciprocal(recip[:], pv[HD:HD + 1, :])
                        rec64 = wpool.tile([HD, QC], f32, tag="rec64")
                        nc.gpsimd.partition_broadcast(rec64[:], recip[:])
                        dst = (outT01[h * 64:(h + 1) * 64, sl] if h < 2
                               else outT2[:, sl])
                        nc.vector.tensor_mul(dst, pv[0:HD, :], rec64[:])
                        if debug and h == 0 and qc == 0:
                            dbgpt = spool.tile([128, KT, QC], f32, tag="dbgpt")
                            nc.vector.tensor_copy(dbgpt[:], ptt[:])
                            nc.sync.dma_start(dbg["pt"][:], dbgpt[:])

                    # partial projection for this q chunk
                    for m in range(6 if "noproj" not in ablate else 0):
                        pp = jpsum.tile([128, QC], f32, tag="pj", name="pp")
                        nc.tensor.matmul(pp[:], pwt0[:, m * 128:(m + 1) * 128],
                                         outT01[:, sl], start=True, stop=False)
                        nc.tensor.matmul(pp[:], pwt1[:, m * 128:(m + 1) * 128],
                                         outT2[:, sl], start=False, stop=True)
                        st = spool.tile([128, QC], f32, tag="stage")
                        nc.vector.tensor_scalar_add(st[:], pp[:], pb[:, m:m + 1])
                        nc.sync.dma_start(d_po[m, qc, :, :], st[:])

                if debug and rep == 0:
                    nc.sync.dma_start(dbg["ot"][0:128, :], outT01[:].bitcast(f32))

    nc.compile()
    return nc


def _get_compiled(debug=False):
    global _cached
    key = ("dbg" if debug else "std")
    if _cached is None:
        _cached = {}
    if key not in _cached:
        mm_dt = os.environ.get("ARP_MM_DT", "float32r")
        pt_dt = os.environ.get("ARP_PT_DT", "float32r")
        reps = int(os.environ.get("ARP_BODY_REPS", "1"))
        _cached[key] = _build_bass(mm_dt, pt_dt, debug=debug, reps=reps)
    return _cached[key]


def _prepare_in_maps(x, qkv_w, proj_w, proj_b, rel_pos_h, rel_pos_w, rel_pos_t):
    x = np.asarray(x, np.float32)
    qkv_w = np.asarray(qkv_w, np.float32)
    proj_w = np.asarray(proj_w, np.float32)
    proj_b = np.asarray(proj_b, np.float32)
    rel_pos_h = np.asarray(rel_pos_h, np.float32)
    rel_pos_w = np.asarray(rel_pos_w, np.float32)
    rel_pos_t = np.asarray(rel_pos_t, np.float32)

    ii = np.arange(HW_)
    rh = 8.0 * rel_pos_h[ii[:, None] - ii[None, :] + (HW_ - 1)]  # [i, j, 64]
    rw = 8.0 * rel_pos_w[ii[:, None] - ii[None, :] + (HW_ - 1)]
    tt = np.arange(T)
    rt = 8.0 * rel_pos_t[tt[:, None] - tt[None, :] + (T - 1)]    # [t, t', 64]
    rht = np.ascontiguousarray(rh.reshape(196, HD).T)            # [64, i*14+j]
    rwt = np.ascontiguousarray(rw.reshape(196, HD).T)
    rtt = np.ascontiguousarray(rt.reshape(64, HD).T)

    aug = np.zeros((NAUG, NK), np.float32)
    k = np.arange(N)
    aug[(k // 14) % 14, k] = 1.0          # onehot_h  (Q' rows 64:78)
    aug[14 + k // S, k] = 1.0             # onehot_t  (Q' rows 78:86)
    aug[32 + k % 14, k] = 1.0             # onehot_w  (Q' rows 96:110; 86:96 pad)

    xt_b = [np.ascontiguousarray(x[b].reshape(N, DIM).T) for b in range(B)]

    cns = np.zeros((128, 110), np.float32)
    cns[:, 96:108] = 1.0
    cns[0:32, 108] = 1.0

    in_maps = []
    for c in range(N_CORES):
        b = c // 4
        heads = [3 * (c % 4) + j for j in range(HEADS_PER_CORE)]
        wcols = []
        for h in heads:
            wcols.append(qkv_w[HD * h:HD * (h + 1), :] * SCALE)       # q
            wcols.append(qkv_w[DIM + HD * h:DIM + HD * (h + 1), :])   # k
        for h in heads:
            wcols.append(qkv_w[2 * DIM + HD * h:2 * DIM + HD * (h + 1), :])
        wt = np.ascontiguousarray(np.concatenate(wcols, axis=0).T)    # [768, 576]
        pcols = np.concatenate([np.arange(HD * h, HD * (h + 1)) for h in heads])
        pwt = np.ascontiguousarray(proj_w[:, pcols].T)                # [192, 768]
        in_maps.append({
            "xt": xt_b[b],
            "wt": wt,
            "rht": rht, "rwt": rwt, "rtt": rtt,
            "aug": aug,
            "pwt": pwt,
            "pb": np.ascontiguousarray(proj_b.reshape(6, 128)),
            "ident": np.eye(128, dtype=np.float32),
            "cns": cns,
        })
    return in_maps


def _unshard(results, dtype):
    out = np.zeros((B, T, S, DIM), dtype)
    for b in range(B):
        acc = results[4 * b]["po"].astype(np.float64)
        for c in range(4 * b + 1, 4 * b + 4):
            acc = acc + results[c]["po"].astype(np.float64)
        # [6, 4, 128, 392] -> [768, 1568] -> transpose to [1568, 768]
        pot = acc.transpose(0, 2, 1, 3).reshape(DIM, N)
        out[b] = pot.T.reshape(T, S, DIM).astype(dtype)
    return out


def kernel(x, qkv_w, proj_w, proj_b, rel_pos_h, rel_pos_w, rel_pos_t):
    from concourse import bass_utils

    debug = bool(int(os.environ.get("ARP_DEBUG", "0")))
    nc = _get_compiled(debug=debug)
    in_maps = _prepare_in_maps(x, qkv_w, proj_w, proj_b,
                               rel_pos_h, rel_pos_w, rel_pos_t)
    res = bass_utils.run_bass_kernel_spmd(nc, in_maps,
                                          core_ids=list(range(N_CORES)))
    kernel._last_results = res.results
    return _unshard(res.results, np.asarray(x).dtype)



# revision 36
# speedup vs baseline: 1.1602x; 1.1602x over previous
"""TRN2 Bass kernel for AttentionRelPos (v2).

Problem: B=2, T=8, S=196 (14x14), DIM=768, HEADS=12, HD=64.
  qkv = x @ qkv_w.T -> q,k,v [B, 12, 1568, 64]
  attn = softmax(q k^T / 8 + decomposed rel-pos bias)
  out = (attn @ v) heads-concat @ proj_w.T + proj_b

Sharding: 24 (batch, head) pairs -> 3 per core (8 cores). Core c handles
batch c//4, heads 3*(c%4)+[0,1,2]. Each core computes a partial final
projection over its 192 channels; the host sums the 4 partials per batch
(tensor-parallel unshard), transposes back and adds proj_b.

Structure (all SBUF operands fp16; PSUM fp32):
  - rel-pos bias folded into QK via augmented contraction dim 100:
    Q'[q] = [q/8, rel_h(q), rel_t(q), rel_w(q)], K'[k] = [k, onehots].
  - exp(softmax) split between the Scalar engine (native Exp) and a
    custom 8-block DVE op computing ((a*x+b)*x+c)^16 ~ e^x (|x|<=3.5).
  - PV uses the score tile as the stationary operand: out[q,65] per
    128-q slice (65 = 64 v-dims + ones column for the denominator),
    so each accumulation step costs 65 PE rows instead of 392.
  - attnout normalized via per-partition reciprocal, transposed back to
    [c, q] with PE transposes for the projection.
  - optional fp8e4 DoubleRow QKV for the q/k projections (ARP_QKV8=1).
"""

import os
import sys

for _p in (
    "/root/.axon_site",
    "/root/.axon_site/_ro/trn_rl_repo",
    "/root/.axon_site/_ro/pypackages",
    "/opt/trn_rl_repo",
):
    if os.path.isdir(_p) and _p not in sys.path:
        sys.path.append(_p)

import numpy as np

B, T, HW_, DIM, HEADS, HD = 2, 8, 14, 768, 12, 64
S = HW_ * HW_          # 196
N = T * S              # 1568
NK = 1664              # key count padded to 13*128
KT = 13                # k tiles of 128
NAUG = 36              # 14 (h) + 8 (t) + 14 (w)
NF = HD + NAUG         # 100 = augmented contraction dim
SCALE = 0.125          # hd ** -0.5
N_CORES = 8
HPC = 3                # heads per core
QCS = (512, 512, 512, 32)   # q chunks
QOF = (0, 512, 1024, 1536)

# exp(x) ~ ((EA*x + EB)*x + EC)**16, max rel err 0.70% on |x| <= 3.5
EA, EB, EC = 0.00194729, 0.06287224, 1.00006965

# const-block column offsets (fp16 columns)
C_WT = 0                      # [128, 6, 384] qk weights
C_WV = C_WT + 6 * 384         # [128, 6, 192] v weights
C_PW1 = C_WV + 6 * 192        # [128, 768] proj rows 0:128
C_PW2 = C_PW1 + 768           # [64, 768]  proj rows 128:192
C_ID = C_PW2 + 768            # [128, 128] identity
C_RH = C_ID + 128             # [64, 196]
C_RW = C_RH + 196             # [64, 196]
C_RT = C_RW + 196             # [64, 64]
C_AUG = C_RT + 64             # [36, NK] at partitions 64:100
CX = C_AUG + NK

_cached = None
_exp_op = None


def _get_exp_op():
    global _exp_op
    if _exp_op is not None:
        return _exp_op
    import concourse.dve_ops as dve_ops
    from concourse.dve_spec import Spec, Src0, C0, C1, C2, sq

    def _exp_ref(in0, in1, s0, s1, imm2):
        return ((in0 * s0 + s1) * in0 + imm2) ** 16

    op = dve_ops.DveOp(
        "EXP_POLY16_ANT",
        Spec(body=sq(sq(sq(sq((Src0 * C0 + C1) * Src0 + C2)))), reference=_exp_ref),
        subdim=False,
        uops_sha={"v3": "b9028a2770b985b4", "v4": "8a0143ec7033f2f1"},
    )
    if op.name not in dve_ops._SUB_OPCODE_FOR_NAME:
        dve_ops.OPS.append(op)
        dve_ops.CUSTOM_DVE_SPECS[op.name] = op.spec
        dve_ops._SUB_OPCODE_FOR_NAME[op.name] = (
            max(dve_ops._SUB_OPCODE_FOR_NAME.values()) + 1
        )
    _exp_op = op
    return op


def _build_bass(qkv8=False):
    ablate = set(os.environ.get("ARP_ABLATE", "").split(","))
    import concourse.bass as bass
    import concourse.mybir as mybir
    import concourse.tile as tile
    from concourse import bacc

    exp_op = _get_exp_op()
    f32 = mybir.dt.float32
    f16 = mybir.dt.float16
    fp8 = mybir.dt.float8e4
    Exp = mybir.ActivationFunctionType.Exp
    Copy = mybir.ActivationFunctionType.Copy
    DR = mybir.MatmulPerfMode.DoubleRow

    nc = bacc.Bacc("TRN2", target_bir_lowering=False, debug=False,
                   num_devices=N_CORES)

    d_xt = nc.dram_tensor("xt", [DIM, N], f16, kind="ExternalInput").ap()
    d_cst = nc.dram_tensor("cst", [128, CX], f16, kind="ExternalInput").ap()
    d_po = nc.dram_tensor("po", [6, 128, N], f16, kind="ExternalOutput").ap()
    if qkv8:
        d_xt8 = nc.dram_tensor("xt8", [DIM, N], fp8, kind="ExternalInput").ap()
        d_wt8 = nc.dram_tensor("wt8", [128, 3, 2, 384], fp8,
                               kind="ExternalInput").ap()

    with tile.TileContext(nc) as tc:
        with (
            tc.tile_pool(name="const", bufs=1) as cpool,
            tc.tile_pool(name="big", bufs=1) as bpool,
        ):
            cst = cpool.tile([128, CX], f16, tag="cst")
            xt = cpool.tile([128, 6, N], f16, tag="xt")
            if qkv8:
                # weights + first x chunks first so QKV can start early
                wt8 = cpool.tile([128, 3, 2, 384], fp8, tag="wt8")
                nc.sync.dma_start(wt8[:], d_wt8[:])
                xt8 = cpool.tile([128, 6, N], fp8, tag="xt8")
                x8r = d_xt8[:].rearrange("(c p) n -> p c n", c=6)
                for c0 in (0, 2, 4):
                    nc.sync.dma_start(xt8[:, c0:c0 + 2, :], x8r[:, c0:c0 + 2, :])
                nc.sync.dma_start(cst[:], d_cst[:])
                xr = d_xt[:].rearrange("(c p) n -> p c n", c=6)
                for c0 in (0, 2, 4):
                    nc.sync.dma_start(xt[:, c0:c0 + 2, :], xr[:, c0:c0 + 2, :])
            else:
                nc.sync.dma_start(cst[:, 0:C_PW1], d_cst[:, 0:C_PW1])
                xr = d_xt[:].rearrange("(c p) n -> p c n", c=6)
                for c0 in range(6):
                    nc.sync.dma_start(xt[:, c0:c0 + 1, :], xr[:, c0:c0 + 1, :])
                nc.sync.dma_start(cst[:, C_PW1:CX], d_cst[:, C_PW1:CX])

            wt = cst[:, C_WT:C_WV].rearrange("p (c x) -> p c x", c=6)
            wv = cst[:, C_WV:C_PW1].rearrange("p (c x) -> p c x", c=6)
            pw1 = cst[:, C_PW1:C_PW2]
            pw2 = cst[0:64, C_PW2:C_ID]
            ident = cst[:, C_ID:C_RH]
            rht = cst[0:64, C_RH:C_RH + 196]
            rwt = cst[0:64, C_RW:C_RW + 196]
            rtt = cst[0:64, C_RT:C_RT + 64]
            aug = cst[64:100, C_AUG:C_AUG + NK]

            qt = bpool.tile([NF, HPC, N], f16, tag="qt")
            kt = bpool.tile([NF, HPC, NK], f16, tag="kt")
            vp = bpool.tile([128, KT, HPC, HD + 1], f16, tag="vp")

            # K' aug rows via SBUF->SBUF DMA; pad zeros via gpsimd memsets
            for h in range(HPC):
                nc.sync.dma_start(kt[HD:NF, h, :], aug)
            nc.gpsimd.memset(kt[0:HD, :, N:NK], 0.0)
            nc.gpsimd.memset(vp[:], 0.0)
            nc.gpsimd.memset(vp[:, 0:KT - 1, :, HD:HD + 1], 1.0)
            nc.gpsimd.memset(vp[0:32, KT - 1, :, HD:HD + 1], 1.0)

            # single PSUM pool pair shared by both phases (a pool close
            # would drain all engines at the phase boundary)
            with (
                tc.tile_pool(name="bigp", bufs=2, space="PSUM") as bigp,
                tc.tile_pool(name="uni", bufs=2, space="PSUM") as upool,
            ):
                # q/k projections: 3 m-tiles [q_h | k_h], 3+1 q chunks
                for mt in range(HPC):
                    for qb0, nqc in ((0, 3), (3, 1)):
                        ps = bigp.tile([128, 3, 512], f32, tag="big")
                        if not qkv8:
                            corder = (os.environ.get("ARP_CORDER", "1") == "1")
                            oloop = ([(c, qi) for c in range(6) for qi in range(nqc)]
                                     if corder else
                                     [(c, qi) for qi in range(nqc) for c in range(6)])
                            for c, qi in oloop:
                                q0, qw = QOF[qb0 + qi], QCS[qb0 + qi]
                                nc.tensor.matmul(
                                    ps[:, qi, 0:qw],
                                    wt[:, c, mt * 128:(mt + 1) * 128],
                                    xt[:, c, q0:q0 + qw],
                                    start=(c == 0), stop=(c == 5),
                                )
                        for qi in range(nqc if qkv8 else 0):
                            q0, qw = QOF[qb0 + qi], QCS[qb0 + qi]
                            if qkv8:
                                for i in range(3):
                                    nc.tensor.matmul(
                                        ps[:, qi, 0:qw],
                                        wt8[:, i, :, mt * 128:(mt + 1) * 128],
                                        xt8[:, 2 * i:2 * i + 2, q0:q0 + qw],
                                        start=(i == 0), stop=(i == 2),
                                        perf_mode=DR,
                                    )
                            else:
                                pass
                        q0 = QOF[qb0]
                        # q scaled by 1/8 here (not in the weights: the fp8
                        # weight copy would underflow e4m3 subnormals)
                        if mt % 2:
                            def eng(d, s):
                                nc.vector.tensor_scalar_mul(d, s, SCALE)
                            eng2 = nc.scalar.copy
                        else:
                            def eng(d, s):
                                nc.scalar.activation(d, s, Copy, scale=SCALE)
                            eng2 = nc.vector.tensor_copy
                        if qb0 == 0:
                            qdst = qt[0:HD, mt, 0:1536].rearrange(
                                "p (a b) -> p a b", a=3)
                            kdst = kt[0:HD, mt, 0:1536].rearrange(
                                "p (a b) -> p a b", a=3)
                            eng(qdst, ps[0:HD])
                            eng2(kdst, ps[HD:128])
                        else:
                            eng(qt[0:HD, mt, 1536:1568], ps[0:HD, 0, 0:32])
                            eng2(kt[0:HD, mt, 1536:1568], ps[HD:128, 0, 0:32])


                # v projection, output directly k-major [n, d]
                for nt in range(KT):
                    n0 = nt * 128
                    nw = 128 if nt < KT - 1 else 32
                    psu = upool.tile([128, 512], f32, tag="u",
                                     name=f"v{nt}")
                    ps = psu[:, 0:192]
                    for c in range(6):
                        nc.tensor.matmul(ps[0:nw, :], xt[:, c, n0:n0 + nw],
                                         wv[:, c, :], start=(c == 0),
                                         stop=(c == 5))
                    dst = vp[0:nw, nt, :, 0:HD]
                    (nc.vector.tensor_copy if nt % 2 else nc.scalar.copy)(
                        dst, ps[0:nw, :].rearrange("p (h d) -> p h d", h=HPC))

                # rel_h / rel_w / rel_t -> Q' aug rows
                qt5 = qt[0:HD, :, :].rearrange("p h (t i w) -> p h t i w",
                                               t=T, i=HW_, w=HW_)
                qtr_h = qt[HD:HD + 14, :, :].rearrange(
                    "p h (t i w) -> p h t i w", t=T, i=HW_, w=HW_)
                qth = qt[HD:HD + 14, :, :].rearrange(
                    "p h (t iw) -> p (h t) iw", iw=196)  # (h t) merged: 24x196
                for i0 in range(0, HW_ if "norel" not in ablate else 0, 2):
                    # pair (i0, i0+1): psum [14, 24, 2, 14], fused single copy
                    psu = bigp.tile([128, 3, 512], f32, tag="big",
                                    name=f"rh{i0}")
                    pr = psu[0:14, 0:2, 0:336].rearrange(
                        "p u (ht w) -> p u ht w", ht=24)
                    for u in range(2):
                        nc.tensor.matmul(pr[:, u, :, :],
                                         rht[:, (i0 + u) * 14:(i0 + u + 1) * 14],
                                         qt5[:, :, :, i0 + u, :],
                                         start=True, stop=True)
                    if os.environ.get("ARP_RELFUSE", "1") == "1":
                        ((nc.vector.tensor_copy if (i0 // 2) % 2 else nc.scalar.copy)(
                            qth[:, :, i0 * 14:(i0 + 2) * 14].rearrange(
                                "p ht (u w) -> p ht u w", u=2),
                            pr[:].rearrange("p u ht w -> p ht u w")))
                    else:
                        for u in range(2):
                            (nc.vector.tensor_copy if u else nc.scalar.copy)(
                                qtr_h[:, :, :, i0 + u, :],
                                pr[:, u, :, :].rearrange(
                                    "p (h t) w -> p h t w", h=HPC))
                # rel_w / rel_t destinations start at partitions 86 / 78,
                # not 32-aligned: engine copies are rejected by the BIR
                # verifier, so bounce via 0-aligned stages + SBUF DMAs.
                stw = bpool.tile([14, HPC, N], f16, tag="stw")
                stw5 = stw[:].rearrange("p h (t i w) -> p h t i w",
                                        t=T, i=HW_, w=HW_)
                for j0 in range(0, HW_ if "norel" not in ablate else 0, 2):
                    psu = bigp.tile([128, 3, 512], f32, tag="big",
                                    name=f"rw{j0}")
                    pr = psu[0:14, 0:2, 0:336].rearrange(
                        "p u (ht i) -> p u ht i", ht=24)
                    for u in range(2):
                        nc.tensor.matmul(pr[:, u, :, :],
                                         rwt[:, (j0 + u) * 14:(j0 + u + 1) * 14],
                                         qt5[:, :, :, :, j0 + u],
                                         start=True, stop=True)
                    # stw free layout (h t i w): copy pair (j0, j0+1) as the
                    # w-positions j0, j0+1 for all (ht, i) -> strided dst
                    if os.environ.get("ARP_RELFUSE", "1") == "1":
                        ((nc.vector.tensor_copy if (j0 // 2) % 2 else nc.scalar.copy)(
                            stw5[:, :, :, :, j0:j0 + 2].rearrange(
                                "p h t i u -> p (h t) u i"),
                            pr[:].rearrange("p u ht i -> p ht u i")))
                    else:
                        for u in range(2):
                            (nc.vector.tensor_copy if u else nc.scalar.copy)(
                                stw5[:, :, :, :, j0 + u],
                                pr[:, u, :, :].rearrange(
                                    "p (h t) i -> p h t i", h=HPC))
                stt = bpool.tile([8, HPC, N], f16, tag="stt")
                for t in range(T if "norel" not in ablate else 0):
                    psu = bigp.tile([128, 3, 512], f32, tag="big",
                                    name=f"rt{t}")
                    pr = psu[0:8, 0:2, 0:294].rearrange(
                        "p u hw -> p u hw")
                    for half in range(2):
                        c0 = t * S + half * 98
                        nc.tensor.matmul(pr[:, half, :],
                                         rtt[:, t * 8:(t + 1) * 8],
                                         qt[0:HD, :, c0:c0 + 98],
                                         start=True, stop=True)
                    if os.environ.get("ARP_RELFUSE", "1") == "1":
                        (nc.vector.tensor_copy if t % 2 else nc.scalar.copy)(
                            stt[:, :, t * S:(t + 1) * S].rearrange(
                                "p h (u w) -> p u h w", u=2),
                            pr[:].rearrange("p u (h w) -> p u h w", h=HPC))
                    else:
                        for half in range(2):
                            c0 = t * S + half * 98
                            (nc.vector.tensor_copy if half else nc.scalar.copy)(
                                stt[:, :, c0:c0 + 98],
                                pr[:, half, :].rearrange("p (h w) -> p h w", h=HPC))
                if "norel" not in ablate:
                    nc.sync.dma_start(qt[HD + 22:NF, :, :], stw[:])
                    nc.sync.dma_start(qt[HD + 14:HD + 22, :, :], stt[:])

                # ---------- phase 2: attention + projection ----------
                ptp_cm = tc.tile_pool(name="ptp", bufs=3)
                aop_cm = tc.tile_pool(name="aop", bufs=2)
                aotp_cm = tc.tile_pool(name="aotp", bufs=1)
                ptp = ptp_cm.__enter__()
                aop = aop_cm.__enter__()
                aotp = aotp_cm.__enter__()
                aoT1 = aotp.tile([128, N], f16, tag="aoT1")
                aoT2 = aotp.tile([64, N], f16, tag="aoT2")
                groups = ((0, 3), (3, 3), (6, 3), (9, 2), (11, 2))
                units = [(qc, h) for qc in range(4) for h in range(HPC)]
                pending = []
                live = {}   # unit idx -> (ptt, pv, rc, ao, diag, qc, h)

                def emit_qk_exp(i):
                    qc, h = units[i]
                    q0, qw = QOF[qc], QCS[qc]
                    ptt = ptp.tile([128, KT, 512], f16, tag="pt",
                                   name=f"pt{i}")
                    for gi, (g0, glen) in enumerate(groups):
                        sp = bigp.tile([128, 3, 512], f32, tag="big")
                        for j in range(glen):
                            k = g0 + j
                            nc.tensor.matmul(
                                sp[:, j, 0:qw],
                                kt[:, h, k * 128:(k + 1) * 128],
                                qt[:, h, q0:q0 + qw],
                                start=True, stop=True,
                            )
                        if "noexp" in ablate:
                            (nc.scalar.copy if gi in (0, 3, 4)
                             else nc.vector.tensor_copy)(
                                ptt[:, g0:g0 + glen, 0:1],
                                sp[:, 0:glen, 0:1])
                        elif gi in (0, 3, 4):
                            nc.scalar.activation(
                                ptt[:, g0:g0 + glen, 0:qw],
                                sp[:, 0:glen, 0:qw], Exp)
                        else:
                            nc.vector._custom_dve(
                                exp_op,
                                out=ptt[:, g0:g0 + glen, 0:qw],
                                in0=sp[:, 0:glen, 0:qw],
                                s0=EA, s1=EB, imm2=EC)
                    live[i] = ptt

                def emit_pv(i):
                    qc, h = units[i]
                    q0, qw = QOF[qc], QCS[qc]
                    nsl = (qw + 127) // 128
                    ptt = live.pop(i)
                    if h == 0:
                        emit_pv.ao = aop.tile([128, 4, HPC, HD], f16,
                                              tag="ao", name=f"ao{qc}")
                        emit_pv.diag = aop.tile([128, 4, HPC, 128], f16,
                                                tag="diag", name=f"dg{qc}")
                    ao, diag = emit_pv.ao, emit_pv.diag
                    pvu = upool.tile([128, 512], f32, tag="u",
                                     name=f"pv{qc}_{h}")
                    pv = pvu[:, 0:260].rearrange("p (s d) -> p s d", s=4)
                    for s in range(nsl):
                        sw = min(128, qw - s * 128)
                        s0 = s * 128
                        for k in range(KT):
                            nc.tensor.matmul(
                                pv[0:sw, s, :],
                                ptt[:, k, s0:s0 + sw],
                                vp[:, k, h, :],
                                start=(k == 0), stop=(k == KT - 1),
                            )
                    rc = aop.tile([128, 4], f32, tag="rc", name=f"rc{qc}_{h}")
                    nc.vector.reciprocal(rc[:, 0:nsl], pv[:, 0:nsl, HD])
                    # raw (unnormalized) attnout; normalization rides the
                    # transpose matmul via a diag(1/D) moving operand
                    (nc.vector.tensor_copy if h % 2 else nc.scalar.copy)(
                        ao[:, 0:nsl, h, :], pv[:, 0:nsl, 0:HD])
                    for s in range(nsl):
                        sw = min(128, qw - s * 128)
                        nc.gpsimd.tensor_scalar_mul(
                            diag[0:sw, s, h, 0:sw], ident[0:sw, 0:sw],
                            rc[0:sw, s:s + 1])
                    if h == HPC - 1:
                        pending.append((qc, ao, diag))

                def emit_tail_a(qc, ao, diag):
                    q0, qw = QOF[qc], QCS[qc]
                    nsl = (qw + 127) // 128
                    # normalize + transpose attnout back to [c, q] via PE
                    for p0 in range(0, nsl, 2):
                        pn = min(2, nsl - p0)
                        tu = upool.tile([128, 512], f32, tag="u",
                                        name=f"t{qc}_{p0}")
                        tA = tu[:, 0:256].rearrange("p (a b) -> p a b", a=2)
                        tB = tu[0:64, 256:512].rearrange(
                            "p (a b) -> p a b", a=2)
                        for j in range(pn):
                            s = p0 + j
                            sw = min(128, qw - s * 128)
                            for h in range(HPC):
                                dst = (tA[h * 64:(h + 1) * 64, j, 0:sw]
                                       if h < 2 else tB[:, j, 0:sw])
                                nc.tensor.matmul(
                                    dst, ao[0:sw, s, h, :],
                                    diag[0:sw, s, h, 0:sw],
                                    start=True, stop=True)
                        c0 = q0 + p0 * 128
                        cw = min(256, qw - p0 * 128)
                        dstA = aoT1[:, c0:c0 + cw]
                        dstB = aoT2[:, c0:c0 + cw]
                        if cw > 128:
                            dstA = dstA.rearrange("p (a b) -> p a b", a=2)
                            dstB = dstB.rearrange("p (a b) -> p a b", a=2)
                            srcA, srcB = tA[:, 0:2, :], tB[:, 0:2, :]
                        else:
                            dstA = dstA[:, None, :]
                            dstB = dstB[:, None, :]
                            srcA, srcB = tA[:, 0:1, 0:cw], tB[:, 0:1, 0:cw]
                        if (p0 // 2) % 2:
                            nc.vector.tensor_copy(dstA, srcA)
                            nc.scalar.copy(dstB, srcB)
                        else:
                            nc.scalar.copy(dstA, srcA)
                            nc.vector.tensor_copy(dstB, srcB)

                def emit_tail_b(qc, part):
                    q0, qw = QOF[qc], QCS[qc]
                    # partial projection for this q chunk (two 3-m slots)
                    if part == 0:
                        emit_tail_b.stg = aop.tile([128, 6, 512], f16,
                                                   tag="stg", name=f"stg{qc}")
                    stg = emit_tail_b.stg
                    for m in range(part * 3, part * 3 + 3):
                        pp = upool.tile([128, 512], f32, tag="u",
                                        name=f"pp{qc}_{m}")
                        nc.tensor.matmul(pp[:, 0:qw],
                                         pw1[:, m * 128:(m + 1) * 128],
                                         aoT1[:, q0:q0 + qw],
                                         start=True, stop=False)
                        nc.tensor.matmul(pp[:, 0:qw],
                                         pw2[:, m * 128:(m + 1) * 128],
                                         aoT2[:, q0:q0 + qw],
                                         start=False, stop=True)
                        (nc.vector.tensor_copy if m % 2 else nc.scalar.copy)(
                            stg[:, m, 0:qw], pp[:, 0:qw])
                    if part == 1:
                        nc.sync.dma_start(
                            d_po[:, :, q0:q0 + qw].rearrange("m p q -> p m q"),
                            stg[:, :, 0:qw])

                # software pipeline: PV lags QK/exp by one unit so the PE
                # never stalls waiting for the exp of its own score tile
                pend_a, pend_b = [], []

                def drain(kind):
                    if kind == "a" and pending:
                        emit_tail_a(*pending.pop(0))
                    elif kind == "b" and pend_b:
                        emit_tail_b(pend_b.pop(0))

                LAG = int(os.environ.get("ARP_LAG", "1"))
                nu = len(units)
                for i in range(nu):
                    emit_qk_exp(i)
                    if i >= LAG:
                        emit_pv(i - LAG)
                    # run deferred per-qc tails one slot later each
                    if pending:
                        qc0 = pending[0][0]
                        emit_tail_a(*pending.pop(0))
                        pend_b.extend([(qc0, 0), (qc0, 1)])
                    elif pend_b:
                        emit_tail_b(*pend_b.pop(0))
                for i in range(nu - LAG, nu):
                    emit_pv(i)
                while pending:
                    qc0 = pending[0][0]
                    emit_tail_a(*pending.pop(0))
                    pend_b.extend([(qc0, 0), (qc0, 1)])
                while pend_b:
                    emit_tail_b(*pend_b.pop(0))
                aotp_cm.__exit__(None, None, None)
                aop_cm.__exit__(None, None, None)
                ptp_cm.__exit__(None, None, None)

    nc.compile()
    return nc


def _get_compiled():
    global _cached
    if _cached is None:
        qkv8 = bool(int(os.environ.get("ARP_QKV8", "0")))
        _cached = _build_bass(qkv8=qkv8)
    return _cached


def _prepare_in_maps(x, qkv_w, proj_w, proj_b, rel_pos_h, rel_pos_w, rel_pos_t,
                     qkv8=False):
    import ml_dtypes
    f16 = np.float16
    x = np.asarray(x, np.float32)
    qkv_w = np.asarray(qkv_w, np.float32)
    proj_w = np.asarray(proj_w, np.float32)

    ii = np.arange(HW_)
    rh = 8.0 * np.asarray(rel_pos_h, np.float32)[ii[:, None] - ii[None, :] + (HW_ - 1)]
    rw = 8.0 * np.asarray(rel_pos_w, np.float32)[ii[:, None] - ii[None, :] + (HW_ - 1)]
    tt = np.arange(T)
    rt = 8.0 * np.asarray(rel_pos_t, np.float32)[tt[:, None] - tt[None, :] + (T - 1)]
    rht = rh.reshape(196, HD).T        # [64, 196]
    rwt = rw.reshape(196, HD).T
    rtt = rt.reshape(64, HD).T         # [64, 64]

    aug = np.zeros((NAUG, NK), np.float32)
    k = np.arange(N)
    aug[(k // 14) % 14, k] = 1.0       # onehot_h  (Q' rows 64:78)
    aug[14 + k // S, k] = 1.0          # onehot_t  (rows 78:86)
    aug[22 + k % 14, k] = 1.0          # onehot_w  (rows 86:100)

    xt_b = [np.ascontiguousarray(x[b].reshape(N, DIM).T) for b in range(B)]

    in_maps = []
    for c in range(N_CORES):
        b = c // 4
        heads = [3 * (c % 4) + j for j in range(HPC)]
        wcols = []
        for h in heads:
            wcols.append(qkv_w[HD * h:HD * (h + 1), :])               # q
            wcols.append(qkv_w[DIM + HD * h:DIM + HD * (h + 1), :])   # k
        wqk = np.concatenate(wcols, axis=0).T                          # [768, 384]
        vcols = [qkv_w[2 * DIM + HD * h:2 * DIM + HD * (h + 1), :] for h in heads]
        wvv = np.concatenate(vcols, axis=0).T                          # [768, 192]
        pcols = np.concatenate([np.arange(HD * h, HD * (h + 1)) for h in heads])
        pwt = proj_w[:, pcols].T                                       # [192, 768]

        cst = np.zeros((128, CX), np.float32)
        cst[:, C_WT:C_WV] = wqk.reshape(6, 128, 384).transpose(1, 0, 2).reshape(128, -1)
        cst[:, C_WV:C_PW1] = wvv.reshape(6, 128, 192).transpose(1, 0, 2).reshape(128, -1)
        cst[:, C_PW1:C_PW2] = pwt[0:128]
        cst[0:64, C_PW2:C_ID] = pwt[128:192]
        cst[:, C_ID:C_RH] = np.eye(128, dtype=np.float32)
        cst[0:64, C_RH:C_RH + 196] = rht
        cst[0:64, C_RW:C_RW + 196] = rwt
        cst[0:64, C_RT:C_RT + 64] = rtt
        cst[64:100, C_AUG:C_AUG + NK] = aug

        m = {
            "xt": xt_b[b].astype(f16),
            "cst": cst.astype(f16),
        }
        if qkv8:
            m["xt8"] = xt_b[b].astype(ml_dtypes.float8_e4m3)
            # [128, 3 cpair, 2, 384]: weight chunk pairs for DoubleRow
            w8 = wqk.reshape(3, 2, 128, 384).transpose(2, 0, 1, 3)
            m["wt8"] = np.ascontiguousarray(w8).astype(ml_dtypes.float8_e4m3)
        in_maps.append(m)
    return in_maps


def _unshard(results, proj_b, dtype):
    proj_b = np.asarray(proj_b, np.float64)
    out = np.zeros((B, T, S, DIM), dtype)
    for b in range(B):
        acc = results[4 * b]["po"].astype(np.float64)
        for c in range(4 * b + 1, 4 * b + 4):
            acc = acc + results[c]["po"].astype(np.float64)
        pot = acc.reshape(DIM, N)          # [6*128, 1568]
        out[b] = (pot.T + proj_b).reshape(T, S, DIM).astype(dtype)
    return out


def kernel(x, qkv_w, proj_w, proj_b, rel_pos_h, rel_pos_w, rel_pos_t):
    from concourse import bass_utils

    qkv8 = bool(int(os.environ.get("ARP_QKV8", "0")))
    nc = _get_compiled()
    in_maps = _prepare_in_maps(x, qkv_w, proj_w, proj_b,
                               rel_pos_h, rel_pos_w, rel_pos_t, qkv8=qkv8)
    res = bass_utils.run_bass_kernel_spmd(nc, in_maps,
                                          core_ids=list(range(N_CORES)))
    kernel._last_results = res.results
    return _unshard(res.results, proj_b, np.asarray(x).dtype)


# revision 37
# speedup vs baseline: 1.2242x; 1.0551x over previous
"""TRN2 Bass kernel for AttentionRelPos (v2).

Problem: B=2, T=8, S=196 (14x14), DIM=768, HEADS=12, HD=64.
  qkv = x @ qkv_w.T -> q,k,v [B, 12, 1568, 64]
  attn = softmax(q k^T / 8 + decomposed rel-pos bias)
  out = (attn @ v) heads-concat @ proj_w.T + proj_b

Sharding: 24 (batch, head) pairs -> 3 per core (8 cores). Core c handles
batch c//4, heads 3*(c%4)+[0,1,2]. Each core computes a partial final
projection over its 192 channels; the host sums the 4 partials per batch
(tensor-parallel unshard), transposes back and adds proj_b.

Structure (all SBUF operands fp16; PSUM fp32):
  - rel-pos bias folded into QK via augmented contraction dim 100:
    Q'[q] = [q/8, rel_h(q), rel_t(q), rel_w(q)], K'[k] = [k, onehots].
  - exp(softmax) split between the Scalar engine (native Exp) and a
    custom 8-block DVE op computing ((a*x+b)*x+c)^16 ~ e^x (|x|<=3.5).
  - PV uses the score tile as the stationary operand: out[q,65] per
    128-q slice (65 = 64 v-dims + ones column for the denominator),
    so each accumulation step costs 65 PE rows instead of 392.
  - attnout normalized via per-partition reciprocal, transposed back to
    [c, q] with PE transposes for the projection.
  - optional fp8e4 DoubleRow QKV for the q/k projections (ARP_QKV8=1).
"""

import os
import sys

for _p in (
    "/root/.axon_site",
    "/root/.axon_site/_ro/trn_rl_repo",
    "/root/.axon_site/_ro/pypackages",
    "/opt/trn_rl_repo",
):
    if os.path.isdir(_p) and _p not in sys.path:
        sys.path.append(_p)

import numpy as np

B, T, HW_, DIM, HEADS, HD = 2, 8, 14, 768, 12, 64
S = HW_ * HW_          # 196
N = T * S              # 1568
NK = 1664              # key count padded to 13*128
KT = 13                # k tiles of 128
NAUG = 36              # 14 (h) + 8 (t) + 14 (w)
NF = HD + NAUG         # 100 = augmented contraction dim
SCALE = 0.125          # hd ** -0.5
N_CORES = 8
HPC = 3                # heads per core
QCS = (512, 512, 512, 32)   # q chunks
QOF = (0, 512, 1024, 1536)

# exp(x) ~ ((EA*x + EB)*x + EC)**16, max rel err 0.70% on |x| <= 3.5
EA, EB, EC = 0.00194729, 0.06287224, 1.00006965

# const-block column offsets (fp16 columns)
C_WT = 0                      # [128, 6, 384] qk weights
C_WV = C_WT + 6 * 384         # [128, 6, 192] v weights
C_PW1 = C_WV + 6 * 192        # [128, 768] proj rows 0:128
C_PW2 = C_PW1 + 768           # [64, 768]  proj rows 128:192
C_ID = C_PW2 + 768            # [128, 128] identity
C_RH = C_ID + 128             # [64, 196]
C_RW = C_RH + 196             # [64, 196]
C_RT = C_RW + 196             # [64, 64]
C_AUG = C_RT + 64             # [36, NK] at partitions 64:100
CX = C_AUG + NK

_cached = None
_exp_op = None


def _get_exp_op():
    global _exp_op
    if _exp_op is not None:
        return _exp_op
    import concourse.dve_ops as dve_ops
    from concourse.dve_spec import Spec, Src0, C0, C1, C2, sq

    def _exp_ref(in0, in1, s0, s1, imm2):
        return ((in0 * s0 + s1) * in0 + imm2) ** 16

    op = dve_ops.DveOp(
        "EXP_POLY16_ANT",
        Spec(body=sq(sq(sq(sq((Src0 * C0 + C1) * Src0 + C2)))), reference=_exp_ref),
        subdim=False,
        uops_sha={"v3": "b9028a2770b985b4", "v4": "8a0143ec7033f2f1"},
    )
    if op.name not in dve_ops._SUB_OPCODE_FOR_NAME:
        dve_ops.OPS.append(op)
        dve_ops.CUSTOM_DVE_SPECS[op.name] = op.spec
        dve_ops._SUB_OPCODE_FOR_NAME[op.name] = (
            max(dve_ops._SUB_OPCODE_FOR_NAME.values()) + 1
        )
    _exp_op = op
    return op


def _build_bass(qkv8=False):
    ablate = set(os.environ.get("ARP_ABLATE", "").split(","))
    import concourse.bass as bass
    import concourse.mybir as mybir
    import concourse.tile as tile
    from concourse import bacc

    exp_op = _get_exp_op()
    f32 = mybir.dt.float32
    f16 = mybir.dt.float16
    fp8 = mybir.dt.float8e4
    Exp = mybir.ActivationFunctionType.Exp
    Copy = mybir.ActivationFunctionType.Copy
    DR = mybir.MatmulPerfMode.DoubleRow

    nc = bacc.Bacc("TRN2", target_bir_lowering=False, debug=False,
                   num_devices=N_CORES)

    d_xt = nc.dram_tensor("xt", [DIM, N], f16, kind="ExternalInput").ap()
    d_cst = nc.dram_tensor("cst", [128, CX], f16, kind="ExternalInput").ap()
    d_po = nc.dram_tensor("po", [6, 128, N], f16, kind="ExternalOutput").ap()
    if qkv8:
        d_xt8 = nc.dram_tensor("xt8", [DIM, N], fp8, kind="ExternalInput").ap()
        d_wt8 = nc.dram_tensor("wt8", [128, 3, 2, 384], fp8,
                               kind="ExternalInput").ap()

    with tile.TileContext(nc) as tc:
        with (
            tc.tile_pool(name="const", bufs=1) as cpool,
            tc.tile_pool(name="big", bufs=1) as bpool,
        ):
            cst = cpool.tile([128, CX], f16, tag="cst")
            xt = cpool.tile([128, 6, N], f16, tag="xt")
            if qkv8:
                # weights + first x chunks first so QKV can start early
                wt8 = cpool.tile([128, 3, 2, 384], fp8, tag="wt8")
                nc.sync.dma_start(wt8[:], d_wt8[:])
                xt8 = cpool.tile([128, 6, N], fp8, tag="xt8")
                x8r = d_xt8[:].rearrange("(c p) n -> p c n", c=6)
                for c0 in (0, 2, 4):
                    nc.sync.dma_start(xt8[:, c0:c0 + 2, :], x8r[:, c0:c0 + 2, :])
                nc.sync.dma_start(cst[:], d_cst[:])
                xr = d_xt[:].rearrange("(c p) n -> p c n", c=6)
                for c0 in (0, 2, 4):
                    nc.sync.dma_start(xt[:, c0:c0 + 2, :], xr[:, c0:c0 + 2, :])
            else:
                nc.sync.dma_start(cst[:, 0:C_WV], d_cst[:, 0:C_WV])
                xr = d_xt[:].rearrange("(c p) n -> p c n", c=6)
                for c0 in range(6):
                    nc.sync.dma_start(xt[:, c0:c0 + 1, :], xr[:, c0:c0 + 1, :])
                nc.sync.dma_start(cst[:, C_WV:CX], d_cst[:, C_WV:CX])

            wt = cst[:, C_WT:C_WV].rearrange("p (c x) -> p c x", c=6)
            wv = cst[:, C_WV:C_PW1].rearrange("p (c x) -> p c x", c=6)
            pw1 = cst[:, C_PW1:C_PW2]
            pw2 = cst[0:64, C_PW2:C_ID]
            ident = cst[:, C_ID:C_RH]
            rht = cst[0:64, C_RH:C_RH + 196]
            rwt = cst[0:64, C_RW:C_RW + 196]
            rtt = cst[0:64, C_RT:C_RT + 64]
            aug = cst[64:100, C_AUG:C_AUG + NK]

            qt = bpool.tile([NF, HPC, N], f16, tag="qt")
            kt = bpool.tile([NF, HPC, NK], f16, tag="kt")
            vp = bpool.tile([128, KT, HPC, HD + 1], f16, tag="vp")

            # K' aug rows via SBUF->SBUF DMA; pad zeros via gpsimd memsets
            for h in range(HPC):
                nc.sync.dma_start(kt[HD:NF, h, :], aug)
            nc.gpsimd.memset(kt[0:HD, :, N:NK], 0.0)
            nc.gpsimd.memset(vp[:], 0.0)
            nc.gpsimd.memset(vp[:, 0:KT - 1, :, HD:HD + 1], 1.0)
            nc.gpsimd.memset(vp[0:32, KT - 1, :, HD:HD + 1], 1.0)

            # single PSUM pool pair shared by both phases (a pool close
            # would drain all engines at the phase boundary)
            with (
                tc.tile_pool(name="bigp", bufs=2, space="PSUM") as bigp,
                tc.tile_pool(name="uni", bufs=2, space="PSUM") as upool,
            ):
                # q/k projections: 3 m-tiles [q_h | k_h], 3+1 q chunks
                for mt in range(HPC):
                    for qb0, nqc in ((0, 3), (3, 1)):
                        ps = bigp.tile([128, 3, 512], f32, tag="big")
                        if not qkv8:
                            corder = (os.environ.get("ARP_CORDER", "1") == "1")
                            oloop = ([(c, qi) for c in range(6) for qi in range(nqc)]
                                     if corder else
                                     [(c, qi) for qi in range(nqc) for c in range(6)])
                            for c, qi in oloop:
                                q0, qw = QOF[qb0 + qi], QCS[qb0 + qi]
                                nc.tensor.matmul(
                                    ps[:, qi, 0:qw],
                                    wt[:, c, mt * 128:(mt + 1) * 128],
                                    xt[:, c, q0:q0 + qw],
                                    start=(c == 0), stop=(c == 5),
                                )
                        for qi in range(nqc if qkv8 else 0):
                            q0, qw = QOF[qb0 + qi], QCS[qb0 + qi]
                            if qkv8:
                                for i in range(3):
                                    nc.tensor.matmul(
                                        ps[:, qi, 0:qw],
                                        wt8[:, i, :, mt * 128:(mt + 1) * 128],
                                        xt8[:, 2 * i:2 * i + 2, q0:q0 + qw],
                                        start=(i == 0), stop=(i == 2),
                                        perf_mode=DR,
                                    )
                            else:
                                pass
                        q0 = QOF[qb0]
                        # q scaled by 1/8 here (not in the weights: the fp8
                        # weight copy would underflow e4m3 subnormals)
                        if mt % 2:
                            def eng(d, s):
                                nc.vector.tensor_scalar_mul(d, s, SCALE)
                            eng2 = nc.scalar.copy
                        else:
                            def eng(d, s):
                                nc.scalar.activation(d, s, Copy, scale=SCALE)
                            eng2 = nc.vector.tensor_copy
                        if qb0 == 0:
                            qdst = qt[0:HD, mt, 0:1536].rearrange(
                                "p (a b) -> p a b", a=3)
                            kdst = kt[0:HD, mt, 0:1536].rearrange(
                                "p (a b) -> p a b", a=3)
                            eng(qdst, ps[0:HD])
                            eng2(kdst, ps[HD:128])
                        else:
                            eng(qt[0:HD, mt, 1536:1568], ps[0:HD, 0, 0:32])
                            eng2(kt[0:HD, mt, 1536:1568], ps[HD:128, 0, 0:32])


                # v projection, output directly k-major [n, d]
                for nt in range(KT):
                    n0 = nt * 128
                    nw = 128 if nt < KT - 1 else 32
                    psu = upool.tile([128, 512], f32, tag="u",
                                     name=f"v{nt}")
                    ps = psu[:, 0:192]
                    for c in range(6):
                        nc.tensor.matmul(ps[0:nw, :], xt[:, c, n0:n0 + nw],
                                         wv[:, c, :], start=(c == 0),
                                         stop=(c == 5))
                    dst = vp[0:nw, nt, :, 0:HD]
                    (nc.vector.tensor_copy if nt % 2 else nc.scalar.copy)(
                        dst, ps[0:nw, :].rearrange("p (h d) -> p h d", h=HPC))

                # rel_h / rel_w / rel_t -> Q' aug rows
                qt5 = qt[0:HD, :, :].rearrange("p h (t i w) -> p h t i w",
                                               t=T, i=HW_, w=HW_)
                qtr_h = qt[HD:HD + 14, :, :].rearrange(
                    "p h (t i w) -> p h t i w", t=T, i=HW_, w=HW_)
                # rel_w / rel_t destinations start at partitions 86 / 78,
                # not 32-aligned: engine copies are rejected by the BIR
                # verifier, so bounce via 0-aligned stages + SBUF DMAs.
                stw = bpool.tile([14, HPC, N], f16, tag="stw")
                stw5 = stw[:].rearrange("p h (t i w) -> p h t i w",
                                        t=T, i=HW_, w=HW_)
                for j0 in range(0, HW_ if "norel" not in ablate else 0, 2):
                    psu = bigp.tile([128, 3, 512], f32, tag="big",
                                    name=f"rw{j0}")
                    pr = psu[0:14, 0:2, 0:336].rearrange(
                        "p u (ht i) -> p u ht i", ht=24)
                    for u in range(2):
                        nc.tensor.matmul(pr[:, u, :, :],
                                         rwt[:, (j0 + u) * 14:(j0 + u + 1) * 14],
                                         qt5[:, :, :, :, j0 + u],
                                         start=True, stop=True)
                    # stw free layout (h t i w): copy pair (j0, j0+1) as the
                    # w-positions j0, j0+1 for all (ht, i) -> strided dst
                    if os.environ.get("ARP_RELFUSE", "1") == "1":
                        ((nc.vector.tensor_copy if (j0 // 2) % 2 else nc.scalar.copy)(
                            stw5[:, :, :, :, j0:j0 + 2].rearrange(
                                "p h t i u -> p (h t) u i"),
                            pr[:].rearrange("p u ht i -> p ht u i")))
                    else:
                        for u in range(2):
                            (nc.vector.tensor_copy if u else nc.scalar.copy)(
                                stw5[:, :, :, :, j0 + u],
                                pr[:, u, :, :].rearrange(
                                    "p (h t) i -> p h t i", h=HPC))
                stt = bpool.tile([8, HPC, N], f16, tag="stt")
                for t in range(T if "norel" not in ablate else 0):
                    psu = bigp.tile([128, 3, 512], f32, tag="big",
                                    name=f"rt{t}")
                    pr = psu[0:8, 0:2, 0:294].rearrange(
                        "p u hw -> p u hw")
                    for half in range(2):
                        c0 = t * S + half * 98
                        nc.tensor.matmul(pr[:, half, :],
                                         rtt[:, t * 8:(t + 1) * 8],
                                         qt[0:HD, :, c0:c0 + 98],
                                         start=True, stop=True)
                    if os.environ.get("ARP_RELFUSE", "1") == "1":
                        (nc.vector.tensor_copy if t % 2 else nc.scalar.copy)(
                            stt[:, :, t * S:(t + 1) * S].rearrange(
                                "p h (u w) -> p u h w", u=2),
                            pr[:].rearrange("p u (h w) -> p u h w", h=HPC))
                    else:
                        for half in range(2):
                            c0 = t * S + half * 98
                            (nc.vector.tensor_copy if half else nc.scalar.copy)(
                                stt[:, :, c0:c0 + 98],
                                pr[:, half, :].rearrange("p (h w) -> p h w", h=HPC))
                if "norel" not in ablate:
                    nc.sync.dma_start(qt[HD + 22:NF, :, :], stw[:])
                    nc.sync.dma_start(qt[HD + 14:HD + 22, :, :], stt[:])

                qth = qt[HD:HD + 14, :, :].rearrange(
                    "p h (t iw) -> p (h t) iw", iw=196)  # (h t) merged: 24x196
                for i0 in range(0, HW_ if "norel" not in ablate else 0, 2):
                    # pair (i0, i0+1): psum [14, 24, 2, 14], fused single copy
                    psu = bigp.tile([128, 3, 512], f32, tag="big",
                                    name=f"rh{i0}")
                    pr = psu[0:14, 0:2, 0:336].rearrange(
                        "p u (ht w) -> p u ht w", ht=24)
                    for u in range(2):
                        nc.tensor.matmul(pr[:, u, :, :],
                                         rht[:, (i0 + u) * 14:(i0 + u + 1) * 14],
                                         qt5[:, :, :, i0 + u, :],
                                         start=True, stop=True)
                    if os.environ.get("ARP_RELFUSE", "1") == "1":
                        ((nc.vector.tensor_copy if (i0 // 2) % 2 else nc.scalar.copy)(
                            qth[:, :, i0 * 14:(i0 + 2) * 14].rearrange(
                                "p ht (u w) -> p ht u w", u=2),
                            pr[:].rearrange("p u ht w -> p ht u w")))
                    else:
                        for u in range(2):
                            (nc.vector.tensor_copy if u else nc.scalar.copy)(
                                qtr_h[:, :, :, i0 + u, :],
                                pr[:, u, :, :].rearrange(
                                    "p (h t) w -> p h t w", h=HPC))

                # ---------- phase 2: attention + projection ----------
                ptp_cm = tc.tile_pool(name="ptp", bufs=3)
                aop_cm = tc.tile_pool(name="aop", bufs=2)
                aotp_cm = tc.tile_pool(name="aotp", bufs=1)
                ptp = ptp_cm.__enter__()
                aop = aop_cm.__enter__()
                aotp = aotp_cm.__enter__()
                aoT1 = aotp.tile([128, N], f16, tag="aoT1")
                aoT2 = aotp.tile([64, N], f16, tag="aoT2")
                groups = ((0, 3), (3, 3), (6, 3), (9, 2), (11, 2))
                units = [(qc, h) for qc in range(4) for h in range(HPC)]
                pending = []
                live = {}   # unit idx -> (ptt, pv, rc, ao, diag, qc, h)

                def emit_qk_exp(i):
                    qc, h = units[i]
                    q0, qw = QOF[qc], QCS[qc]
                    ptt = ptp.tile([128, KT, 512], f16, tag="pt",
                                   name=f"pt{i}")
                    for gi, (g0, glen) in enumerate(groups):
                        sp = bigp.tile([128, 3, 512], f32, tag="big")
                        for j in range(glen):
                            k = g0 + j
                            nc.tensor.matmul(
                                sp[:, j, 0:qw],
                                kt[:, h, k * 128:(k + 1) * 128],
                                qt[:, h, q0:q0 + qw],
                                start=True, stop=True,
                            )
                        if "noexp" in ablate:
                            (nc.scalar.copy if gi in (0, 3, 4)
                             else nc.vector.tensor_copy)(
                                ptt[:, g0:g0 + glen, 0:1],
                                sp[:, 0:glen, 0:1])
                        elif gi in (0, 3, 4):
                            nc.scalar.activation(
                                ptt[:, g0:g0 + glen, 0:qw],
                                sp[:, 0:glen, 0:qw], Exp)
                        else:
                            nc.vector._custom_dve(
                                exp_op,
                                out=ptt[:, g0:g0 + glen, 0:qw],
                                in0=sp[:, 0:glen, 0:qw],
                                s0=EA, s1=EB, imm2=EC)
                    live[i] = ptt

                def emit_pv(i):
                    qc, h = units[i]
                    q0, qw = QOF[qc], QCS[qc]
                    nsl = (qw + 127) // 128
                    ptt = live.pop(i)
                    if h == 0:
                        emit_pv.ao = aop.tile([128, 4, HPC, HD], f16,
                                              tag="ao", name=f"ao{qc}")
                        emit_pv.diag = aop.tile([128, 4, HPC, 128], f16,
                                                tag="diag", name=f"dg{qc}")
                    ao, diag = emit_pv.ao, emit_pv.diag
                    pvu = upool.tile([128, 512], f32, tag="u",
                                     name=f"pv{qc}_{h}")
                    pv = pvu[:, 0:260].rearrange("p (s d) -> p s d", s=4)
                    for s in range(nsl):
                        sw = min(128, qw - s * 128)
                        s0 = s * 128
                        for k in range(KT):
                            nc.tensor.matmul(
                                pv[0:sw, s, :],
                                ptt[:, k, s0:s0 + sw],
                                vp[:, k, h, :],
                                start=(k == 0), stop=(k == KT - 1),
                            )
                    rc = aop.tile([128, 4], f32, tag="rc", name=f"rc{qc}_{h}")
                    nc.vector.reciprocal(rc[:, 0:nsl], pv[:, 0:nsl, HD])
                    # raw (unnormalized) attnout; normalization rides the
                    # transpose matmul via a diag(1/D) moving operand
                    (nc.vector.tensor_copy if h % 2 else nc.scalar.copy)(
                        ao[:, 0:nsl, h, :], pv[:, 0:nsl, 0:HD])
                    for s in range(nsl):
                        sw = min(128, qw - s * 128)
                        nc.gpsimd.tensor_scalar_mul(
                            diag[0:sw, s, h, 0:sw], ident[0:sw, 0:sw],
                            rc[0:sw, s:s + 1])
                    if h == HPC - 1:
                        pending.append((qc, ao, diag))

                def emit_tail_a(qc, ao, diag):
                    q0, qw = QOF[qc], QCS[qc]
                    nsl = (qw + 127) // 128
                    # normalize + transpose attnout back to [c, q] via PE
                    for p0 in range(0, nsl, 2):
                        pn = min(2, nsl - p0)
                        tu = upool.tile([128, 512], f32, tag="u",
                                        name=f"t{qc}_{p0}")
                        tA = tu[:, 0:256].rearrange("p (a b) -> p a b", a=2)
                        tB = tu[0:64, 256:512].rearrange(
                            "p (a b) -> p a b", a=2)
                        for j in range(pn):
                            s = p0 + j
                            sw = min(128, qw - s * 128)
                            for h in range(HPC):
                                dst = (tA[h * 64:(h + 1) * 64, j, 0:sw]
                                       if h < 2 else tB[:, j, 0:sw])
                                nc.tensor.matmul(
                                    dst, ao[0:sw, s, h, :],
                                    diag[0:sw, s, h, 0:sw],
                                    start=True, stop=True)
                        c0 = q0 + p0 * 128
                        cw = min(256, qw - p0 * 128)
                        dstA = aoT1[:, c0:c0 + cw]
                        dstB = aoT2[:, c0:c0 + cw]
                        if cw > 128:
                            dstA = dstA.rearrange("p (a b) -> p a b", a=2)
                            dstB = dstB.rearrange("p (a b) -> p a b", a=2)
                            srcA, srcB = tA[:, 0:2, :], tB[:, 0:2, :]
                        else:
                            dstA = dstA[:, None, :]
                            dstB = dstB[:, None, :]
                            srcA, srcB = tA[:, 0:1, 0:cw], tB[:, 0:1, 0:cw]
                        if (p0 // 2) % 2:
                            nc.vector.tensor_copy(dstA, srcA)
                            nc.scalar.copy(dstB, srcB)
                        else:
                            nc.scalar.copy(dstA, srcA)
                            nc.vector.tensor_copy(dstB, srcB)

                def emit_tail_b(qc, part):
                    q0, qw = QOF[qc], QCS[qc]
                    # partial projection for this q chunk (two 3-m slots)
                    if part == 0:
                        emit_tail_b.stg = aop.tile([128, 6, 512], f16,
                                                   tag="stg", name=f"stg{qc}")
                    stg = emit_tail_b.stg
                    for m in range(part * 3, part * 3 + 3):
                        pp = upool.tile([128, 512], f32, tag="u",
                                        name=f"pp{qc}_{m}")
                        nc.tensor.matmul(pp[:, 0:qw],
                                         pw1[:, m * 128:(m + 1) * 128],
                                         aoT1[:, q0:q0 + qw],
                                         start=True, stop=False)
                        nc.tensor.matmul(pp[:, 0:qw],
                                         pw2[:, m * 128:(m + 1) * 128],
                                         aoT2[:, q0:q0 + qw],
                                         start=False, stop=True)
                        (nc.vector.tensor_copy if m % 2 else nc.scalar.copy)(
                            stg[:, m, 0:qw], pp[:, 0:qw])
                    if part == 1:
                        nc.sync.dma_start(
                            d_po[:, :, q0:q0 + qw].rearrange("m p q -> p m q"),
                            stg[:, :, 0:qw])

                # software pipeline: PV lags QK/exp by one unit so the PE
                # never stalls waiting for the exp of its own score tile
                pend_a, pend_b = [], []

                def drain(kind):
                    if kind == "a" and pending:
                        emit_tail_a(*pending.pop(0))
                    elif kind == "b" and pend_b:
                        emit_tail_b(pend_b.pop(0))

                LAG = int(os.environ.get("ARP_LAG", "1"))
                nu = len(units)
                for i in range(nu):
                    emit_qk_exp(i)
                    if i >= LAG:
                        emit_pv(i - LAG)
                    # run deferred per-qc tails one slot later each
                    if pending:
                        qc0 = pending[0][0]
                        emit_tail_a(*pending.pop(0))
                        pend_b.extend([(qc0, 0), (qc0, 1)])
                    elif pend_b:
                        emit_tail_b(*pend_b.pop(0))
                for i in range(nu - LAG, nu):
                    emit_pv(i)
                while pending:
                    qc0 = pending[0][0]
                    emit_tail_a(*pending.pop(0))
                    pend_b.extend([(qc0, 0), (qc0, 1)])
                while pend_b:
                    emit_tail_b(*pend_b.pop(0))
                aotp_cm.__exit__(None, None, None)
                aop_cm.__exit__(None, None, None)
                ptp_cm.__exit__(None, None, None)

    nc.compile()
    return nc


def _get_compiled():
    global _cached
    if _cached is None:
        qkv8 = bool(int(os.environ.get("ARP_QKV8", "0")))
        _cached = _build_bass(qkv8=qkv8)
    return _cached


def _prepare_in_maps(x, qkv_w, proj_w, proj_b, rel_pos_h, rel_pos_w, rel_pos_t,
                     qkv8=False):
    import ml_dtypes
    f16 = np.float16
    x = np.asarray(x, np.float32)
    qkv_w = np.asarray(qkv_w, np.float32)
    proj_w = np.asarray(proj_w, np.float32)

    ii = np.arange(HW_)
    rh = 8.0 * np.asarray(rel_pos_h, np.float32)[ii[:, None] - ii[None, :] + (HW_ - 1)]
    rw = 8.0 * np.asarray(rel_pos_w, np.float32)[ii[:, None] - ii[None, :] + (HW_ - 1)]
    tt = np.arange(T)
    rt = 8.0 * np.asarray(rel_pos_t, np.float32)[tt[:, None] - tt[None, :] + (T - 1)]
    rht = rh.reshape(196, HD).T        # [64, 196]
    rwt = rw.reshape(196, HD).T
    rtt = rt.reshape(64, HD).T         # [64, 64]

    aug = np.zeros((NAUG, NK), np.float32)
    k = np.arange(N)
    aug[(k // 14) % 14, k] = 1.0       # onehot_h  (Q' rows 64:78)
    aug[14 + k // S, k] = 1.0          # onehot_t  (rows 78:86)
    aug[22 + k % 14, k] = 1.0          # onehot_w  (rows 86:100)

    xt_b = [np.ascontiguousarray(x[b].reshape(N, DIM).T) for b in range(B)]

    in_maps = []
    for c in range(N_CORES):
        b = c // 4
        heads = [3 * (c % 4) + j for j in range(HPC)]
        wcols = []
        for h in heads:
            wcols.append(qkv_w[HD * h:HD * (h + 1), :])               # q
            wcols.append(qkv_w[DIM + HD * h:DIM + HD * (h + 1), :])   # k
        wqk = np.concatenate(wcols, axis=0).T                          # [768, 384]
        vcols = [qkv_w[2 * DIM + HD * h:2 * DIM + HD * (h + 1), :] for h in heads]
        wvv = np.concatenate(vcols, axis=0).T                          # [768, 192]
        pcols = np.concatenate([np.arange(HD * h, HD * (h + 1)) for h in heads])
        pwt = proj_w[:, pcols].T                                       # [192, 768]

        cst = np.zeros((128, CX), np.float32)
        cst[:, C_WT:C_WV] = wqk.reshape(6, 128, 384).transpose(1, 0, 2).reshape(128, -1)
        cst[:, C_WV:C_PW1] = wvv.reshape(6, 128, 192).transpose(1, 0, 2).reshape(128, -1)
        cst[:, C_PW1:C_PW2] = pwt[0:128]
        cst[0:64, C_PW2:C_ID] = pwt[128:192]
        cst[:, C_ID:C_RH] = np.eye(128, dtype=np.float32)
        cst[0:64, C_RH:C_RH + 196] = rht
        cst[0:64, C_RW:C_RW + 196] = rwt
        cst[0:64, C_RT:C_RT + 64] = rtt
        cst[64:100, C_AUG:C_AUG + NK] = aug

        m = {
            "xt": xt_b[b].astype(f16),
            "cst": cst.astype(f16),
        }
        if qkv8:
            m["xt8"] = xt_b[b].astype(ml_dtypes.float8_e4m3)
            # [128, 3 cpair, 2, 384]: weight chunk pairs for DoubleRow
            w8 = wqk.reshape(3, 2, 128, 384).transpose(2, 0, 1, 3)
            m["wt8"] = np.ascontiguousarray(w8).astype(ml_dtypes.float8_e4m3)
        in_maps.append(m)
    return in_maps


def _unshard(results, proj_b, dtype):
    proj_b = np.asarray(proj_b, np.float64)
    out = np.zeros((B, T, S, DIM), dtype)
    for b in range(B):
        acc = results[4 * b]["po"].astype(np.float64)
        for c in range(4 * b + 1, 4 * b + 4):
            acc = acc + results[c]["po"].astype(np.float64)
        pot = acc.reshape(DIM, N)          # [6*128, 1568]
        out[b] = (pot.T + proj_b).reshape(T, S, DIM).astype(dtype)
    return out


def kernel(x, qkv_w, proj_w, proj_b, rel_pos_h, rel_pos_w, rel_pos_t):
    from concourse import bass_utils

    qkv8 = bool(int(os.environ.get("ARP_QKV8", "0")))
    nc = _get_compiled()
    in_maps = _prepare_in_maps(x, qkv_w, proj_w, proj_b,
                               rel_pos_h, rel_pos_w, rel_pos_t, qkv8=qkv8)
    res = bass_utils.run_bass_kernel_spmd(nc, in_maps,
                                          core_ids=list(range(N_CORES)))
    kernel._last_results = res.results
    return _unshard(res.results, proj_b, np.asarray(x).dtype)
